# revision 28
# baseline (speedup 1.0000x reference)
"""Causal self-attention (B=2, S=2048, D=2048, H=16, Hd=128) on 8 trn2 cores.

Sharding: DP=2 over batch x TP=4 over heads. Core c handles batch c//4 and
global heads [4t, 4t+4) with t = c%4.

Per-core pipeline (one SPMD program):
  A) QKV projection, f32r matmuls: qT/kT produced in (hd, seq) layout bf16,
     v in (seq, hd) layout bf16 (via PE transpose).
  B) Attention, loop qi (q-block of 128) outer / head inner:
     scores in PSUM; exp WITHOUT max-subtraction (scores are O(1): the qk dot
     over 128 dims cannot overflow fp32 exp), row-sums via activation
     accum_out; masked diagonal cols zeroed in P; P scaled by 1/l;
     P transposed on PE (bf16, batched into 512-wide PSUM tiles),
     P^T @ V accumulated -> outT (hd, seq) bf16.
  C) AllGather (groups of 4 cores) per (head, seq-half) in bf16, so the
     first-half gathers and the first half of the projection overlap the
     second half of attention.
  D) Output projection, bf16: y^T (512-col slice, seq) = woT^T @ gathered,
     + bias f32, DMA out.

Host side: shard/transpose inputs with numpy, assemble y from per-core y^T.
"""

import math
from contextlib import ExitStack

import numpy as np
import ml_dtypes

BF16_NP = ml_dtypes.bfloat16

import concourse.bass as bass
import concourse.mybir as mybir
import concourse.tile as tile
from concourse import bacc
from concourse.bass_utils import run_bass_kernel_spmd
from concourse.masks import make_identity

FP32 = mybir.dt.float32
FP32R = mybir.dt.float32r
BF16 = mybir.dt.bfloat16

N_CORES = 8
TP = 4  # tensor-parallel group size (heads)
HPC = 4  # heads per core
B, S, D = 2, 2048, 2048
HD = 128
NB = S // 128  # 16 seq blocks
C_SCALE = 1.0 / math.sqrt(HD)
RG = [[0, 1, 2, 3], [4, 5, 6, 7]]

_NC_CACHE = {}


def build_nc(reps: int = 1, fake_collective: bool = False):
    key = (reps, fake_collective)
    if key in _NC_CACHE:
        return _NC_CACHE[key]
    nc = bacc.Bacc("TRN2", target_bir_lowering=False, debug=False, num_devices=N_CORES)

    xT_d = nc.declare_dram_parameter("xT", [D, S], BF16, isOutput=False)
    wqkT_d = nc.declare_dram_parameter("wqkT", [D, 2 * HPC * HD], BF16, isOutput=False)
    wvT_d = nc.declare_dram_parameter("wvT", [D, HPC * HD], BF16, isOutput=False)
    bqk_d = nc.declare_dram_parameter("bqk", [128, 2 * HPC], FP32, isOutput=False)
    bv_d = nc.declare_dram_parameter("bv", [128, HPC], FP32, isOutput=False)
    woT_d = nc.declare_dram_parameter("woT", [D, HPC * HD], BF16, isOutput=False)
    bo_d = nc.declare_dram_parameter("bo", [128, HPC], FP32, isOutput=False)
    y_t_d = nc.declare_dram_parameter("y_t", [HPC * HD, S], FP32, isOutput=True)

    with tile.TileContext(nc, num_cores=N_CORES) as tc, ExitStack() as octx:
        cpool = octx.enter_context(tc.tile_pool(name="const", bufs=1))
        ident = cpool.tile([128, 128], BF16, tag="ident", name="ident")
        make_identity(nc, ident[:])
        tri_neg = cpool.tile([128, 128], BF16, tag="tri_neg", name="tri_neg")
        nc.gpsimd.memset(tri_neg[:], 0.0)
        # keep 0 where j <= p (visible), else fill -1e30 (masked)
        nc.gpsimd.affine_select(
            out=tri_neg[:], in_=tri_neg[:], pattern=[[-1, 128]],
            compare_op=mybir.AluOpType.is_ge, fill=-1e30, base=0, channel_multiplier=1,
        )
        bqk_sb = cpool.tile([128, 2 * HPC], FP32, tag="bqk", name="bqk")
        nc.sync.dma_start(out=bqk_sb[:], in_=bqk_d[:])
        bv_sb = cpool.tile([128, HPC], FP32, tag="bv", name="bv")
        nc.sync.dma_start(out=bv_sb[:], in_=bv_d[:])
        bo_sb = cpool.tile([128, HPC], FP32, tag="bo", name="bo")
        nc.sync.dma_start(out=bo_sb[:], in_=bo_d[:])

        for rep in range(reps):
            sfx = f"r{rep}"
            # per (head, seq-quarter) gather tensors, bf16
            cc_in = [[nc.dram_tensor(f"cc_in{h}_{s}_{sfx}", [HD, S // 4], BF16)
                      for s in range(4)] for h in range(HPC)]
            cc_out = [[nc.dram_tensor(f"cc_out{h}_{s}_{sfx}", [TP * HD, S // 4], BF16)
                       for s in range(4)] for h in range(HPC)]
            _body(nc, tc, xT_d, wqkT_d, wvT_d, woT_d, y_t_d,
                  bqk_sb, bv_sb, bo_sb, ident, tri_neg, cc_in, cc_out,
                  fake_collective)

    nc.compile()
    _NC_CACHE[key] = nc
    return nc


def _gather(nc, cc_in_t, cc_out_t, src_ap, fake):
    nc.sync.dma_start(out=cc_in_t[:], in_=src_ap)
    if fake:
        for rr in range(TP):
            nc.sync.dma_start(
                out=cc_out_t[rr * HD:(rr + 1) * HD, :], in_=cc_in_t[:])
    else:
        nc.gpsimd.collective_compute(
            "AllGather", mybir.AluOpType.bypass, replica_groups=RG,
            ins=[cc_in_t[:]], outs=[cc_out_t[:]])


def _body(nc, tc, xT_d, wqkT_d, wvT_d, woT_d, y_t_d,
          bqk_sb, bv_sb, bo_sb, ident, tri_neg, cc_in, cc_out,
          fake_collective=False):
    """Single software-pipelined loop over 512-col seq chunks: QKV projection
    for chunk n feeds attention for q-blocks [4n, 4n+4), whose per-quarter
    gathers feed the (program-order-later, scheduler-overlapped) output
    projection."""
    with ExitStack() as ctx:
        qkv_pool = ctx.enter_context(tc.tile_pool(name="qkv", bufs=1))
        # qT/kT per local head: (hd=128, S) bf16;  m 0-3 = q heads, 4-7 = k heads
        qkT_sb = [qkv_pool.tile([128, S], BF16, tag=f"qk{m}", name=f"qk{m}")
                  for m in range(8)]
        # v per local head: (seq-within-block=128, 16 blocks * 128) bf16
        vh_sb = [qkv_pool.tile([128, S], BF16, tag=f"vh{h}", name=f"vh{h}")
                 for h in range(HPC)]
        outT = [qkv_pool.tile([128, S], BF16, tag=f"outT{h}", name=f"outT{h}")
                for h in range(HPC)]

        wA = ctx.enter_context(tc.tile_pool(name="wA", bufs=1))
        wqk_sb = [wA.tile([128, 2 * HPC * HD], BF16, tag=f"wqk{kc}",
                          name=f"wqk{kc}") for kc in range(16)]
        wv_sb = [wA.tile([128, HPC * HD], BF16, tag=f"wv{kc}",
                         name=f"wv{kc}") for kc in range(16)]
        wo_sb = [wA.tile([128, HPC * HD], BF16, tag=f"wo{kc}", name=f"wo{kc}")
                 for kc in range(16)]

        xpool = ctx.enter_context(tc.tile_pool(name="xA", bufs=18))
        vtpool = ctx.enter_context(tc.tile_pool(name="vt", bufs=3))
        ppool = ctx.enter_context(tc.tile_pool(name="P", bufs=3))
        ptpool = ctx.enter_context(tc.tile_pool(name="pt", bufs=4))
        stat = ctx.enter_context(tc.tile_pool(name="stat", bufs=8))
        gpool = ctx.enter_context(tc.tile_pool(name="gD", bufs=17))
        ypool = ctx.enter_context(tc.tile_pool(name="yD", bufs=2))

        psW = ctx.enter_context(tc.tile_pool(name="psW", bufs=5, space="PSUM"))
        psA = psS = psD = psW  # all (128,512) f32 tiles share 4 rotating banks
        psT2 = ctx.enter_context(tc.tile_pool(name="psT2", bufs=2, space="PSUM"))
        psPV = ctx.enter_context(tc.tile_pool(name="psPV", bufs=1, space="PSUM"))

        def attention(h, qi):
            nfull = qi * 128  # cols before the diagonal block
            L = nfull + 128
            P = ppool.tile([128, L], BF16, tag="P", name="P")
            q_blk = qkT_sb[h][:, qi * 128:(qi + 1) * 128]

            ls_parts = []
            col = 0
            while col < L:
                w = min(512, L - col)
                St = psS.tile([128, w], FP32, tag="w512", name="S", padded_shape=[128, 512])
                nc.tensor.matmul(
                    St[:], q_blk, qkT_sb[HPC + h][:, col:col + w],
                    start=True, stop=(col + w <= nfull), skip_group_check=True)
                if col + w > nfull:  # chunk contains diagonal block
                    vis = nfull - col
                    # accumulate ident.T @ tri_neg = tri_neg on PE
                    nc.tensor.matmul(
                        St[:, vis:vis + 128], ident[:], tri_neg[:],
                        start=False, stop=True, skip_group_check=True)
                ls = stat.tile([128, 1], FP32, tag="ls", name="ls")
                nc.scalar.activation(
                    P[:, col:col + w], St[:],
                    mybir.ActivationFunctionType.Exp,
                    bias=0.0, scale=C_SCALE, accum_out=ls[:])
                ls_parts.append(ls)
                col += w

            lt = ls_parts[0]
            for k, extra in enumerate(ls_parts[1:]):
                lt2 = stat.tile([128, 1], FP32, tag=f"lt{k}", name=f"lt{k}")
                nc.vector.tensor_add(lt2[:], lt[:], extra[:])
                lt = lt2
            rinv = stat.tile([128, 1], FP32, tag="rinv", name="rinv")
            nc.vector.reciprocal(rinv[:], lt[:])

            pv = psPV.tile([128, 128], FP32, tag="pv", name="pv")
            nblk = qi + 1
            for g0 in range(0, nblk, 4):
                gn = min(4, nblk - g0)
                nc.vector.tensor_scalar_mul(
                    P[:, g0 * 128:(g0 + gn) * 128],
                    P[:, g0 * 128:(g0 + gn) * 128], rinv[:])
                tps = psT2.tile([128, 512], BF16, tag="tp2", name="tp2")
                for jj in range(gn):
                    nc.tensor.transpose(
                        tps[:, jj * 128:(jj + 1) * 128],
                        P[:, (g0 + jj) * 128:(g0 + jj + 1) * 128],
                        ident[:])
                ptsb = ptpool.tile([128, 512], BF16, tag="pt", name="pt")
                nc.vector.tensor_copy(ptsb[:, :gn * 128], tps[:, :gn * 128])
                for jj in range(gn):
                    j = g0 + jj
                    nc.tensor.matmul(
                        pv[:], vh_sb[h][:, j * 128:(j + 1) * 128],
                        ptsb[:, jj * 128:(jj + 1) * 128],
                        start=(j == 0), stop=(j == qi))
            nc.vector.tensor_copy(outT[h][:, qi * 128:(qi + 1) * 128], pv[:])

        for n in range(4):  # seq chunks of 512
            ncol = slice(n * 512, (n + 1) * 512)
            xts = []
            for kc in range(16):
                # interleave weight loads with the first x pass so the PE can
                # start as soon as the first chunks land
                if n == 0:
                    nc.sync.dma_start(
                        out=wqk_sb[kc][:], in_=wqkT_d[kc * 128:(kc + 1) * 128, :])
                xt = xpool.tile([128, 512], BF16, tag="xt", name="xt")
                nc.sync.dma_start(
                    out=xt[:], in_=xT_d[kc * 128:(kc + 1) * 128, ncol])
                xts.append(xt)
            if n == 0:
                for kc in range(16):
                    nc.sync.dma_start(
                        out=wv_sb[kc][:], in_=wvT_d[kc * 128:(kc + 1) * 128, :])
                for kc in range(16):
                    nc.sync.dma_start(
                        out=wo_sb[kc][:], in_=woT_d[kc * 128:(kc + 1) * 128, :])

            for m in range(12):
                psm = psA.tile([128, 512], FP32, tag="w512", name="psA")
                for kc in range(16):
                    if m < 8:
                        lhsT = wqk_sb[kc][:, m * 128:(m + 1) * 128]
                    else:
                        lhsT = wv_sb[kc][:, (m - 8) * 128:(m - 7) * 128]
                    nc.tensor.matmul(psm[:], lhsT, xts[kc][:],
                                     start=(kc == 0), stop=(kc == 15))
                if m < 8:
                    nc.vector.tensor_scalar_add(
                        qkT_sb[m][:, ncol], psm[:], bqk_sb[:, m:m + 1])
                else:
                    h = m - 8
                    vt = vtpool.tile([128, 512], BF16, tag="vt", name="vt")
                    nc.vector.tensor_scalar_add(
                        vt[:], psm[:], bv_sb[:, h:h + 1])
                    tps = psT2.tile([128, 512], BF16, tag="tp2", name="tp2")
                    for j in range(4):
                        nc.tensor.transpose(
                            tps[:, j * 128:(j + 1) * 128],
                            vt[:, j * 128:(j + 1) * 128], ident[:])
                    nc.vector.tensor_copy(vh_sb[h][:, ncol], tps[:])

            for h in range(HPC):
                for qi in range(4 * n, 4 * n + 4):
                    attention(h, qi)
                _gather(nc, cc_in[h][n], cc_out[h][n],
                        outT[h][:, n * 512:(n + 1) * 512], fake_collective)

        # ---- output projection (scheduler overlaps with later chunks) ----
        with nc.named_scope("out_proj"):
            for n in range(4):
                ncol_out = slice(n * 512, (n + 1) * 512)
                gts = []
                for kc in range(16):
                    gt = gpool.tile([128, 512], BF16, tag="gt", name="gt")
                    nc.sync.dma_start(
                        out=gt[:],
                        in_=cc_out[kc // 4][n][(kc % 4) * 128:(kc % 4 + 1) * 128, :])
                    gts.append(gt)
                for m in range(4):
                    psy = psD.tile([128, 512], FP32, tag="w512", name="py")
                    for kc in range(16):
                        nc.tensor.matmul(
                            psy[:], wo_sb[kc][:, m * 128:(m + 1) * 128],
                            gts[kc][:], start=(kc == 0), stop=(kc == 15))
                    yt = ypool.tile([128, 512], FP32, tag="yt", name="yt")
                    nc.scalar.activation(
                        yt[:], psy[:],
                        mybir.ActivationFunctionType.Identity,
                        bias=bo_sb[:, m:m + 1], scale=1.0)
                    nc.sync.dma_start(
                        out=y_t_d[m * 128:(m + 1) * 128, ncol_out], in_=yt[:])


def make_in_maps(x, w_qkv, b_qkv, w_out, b_out):
    in_maps = []
    # gathered row g = h*512 + r*128 + i  <->  w_out column (4r+h)*128 + i
    dorder = np.array(
        [(4 * r + h) * 128 + i for h in range(HPC) for r in range(TP)
         for i in range(HD)])
    for c in range(N_CORES):
        b, t = divmod(c, TP)
        xT = np.ascontiguousarray(x[b].T)
        wq = w_qkv[512 * t:512 * (t + 1)]
        wk = w_qkv[D + 512 * t:D + 512 * (t + 1)]
        wv = w_qkv[2 * D + 512 * t:2 * D + 512 * (t + 1)]
        wqkT = np.ascontiguousarray(np.concatenate([wq, wk], axis=0).T)
        wvT = np.ascontiguousarray(wv.T)
        offs_qk = [512 * t + hh * 128 for hh in range(4)] + \
                  [D + 512 * t + hh * 128 for hh in range(4)]
        bqk = np.stack([b_qkv[o:o + 128] for o in offs_qk], axis=1)
        bv = np.stack(
            [b_qkv[2 * D + 512 * t + hh * 128:2 * D + 512 * t + hh * 128 + 128]
             for hh in range(4)], axis=1)
        woT = np.ascontiguousarray(w_out[512 * t:512 * (t + 1)][:, dorder].T)
        bo = np.ascontiguousarray(b_out[512 * t:512 * (t + 1)].reshape(4, 128).T)
        in_maps.append(dict(
            xT=xT.astype(BF16_NP), wqkT=wqkT.astype(BF16_NP),
            wvT=wvT.astype(BF16_NP),
            bqk=np.ascontiguousarray(bqk), bv=np.ascontiguousarray(bv),
            woT=woT.astype(BF16_NP), bo=bo))
    return in_maps


def assemble_y(results):
    y = np.empty((B, S, D), np.float32)
    for c in range(N_CORES):
        b, t = divmod(c, TP)
        y[b][:, 512 * t:512 * (t + 1)] = results[c]["y_t"].T
    return y


def kernel(x, w_qkv, b_qkv, w_out, b_out):
    x = np.asarray(x, dtype=np.float32)
    w_qkv = np.asarray(w_qkv, dtype=np.float32)
    b_qkv = np.asarray(b_qkv, dtype=np.float32)
    w_out = np.asarray(w_out, dtype=np.float32)
    b_out = np.asarray(b_out, dtype=np.float32)

    nc = build_nc(1)
    in_maps = make_in_maps(x, w_qkv, b_qkv, w_out, b_out)
    r = run_bass_kernel_spmd(nc, in_maps, list(range(N_CORES)))
    return assemble_y(r.results)


# revision 29
# speedup vs baseline: 1.0358x; 1.0358x over previous
"""Causal self-attention (B=2, S=2048, D=2048, H=16, Hd=128) on 8 trn2 cores.

Sharding: DP=2 over batch x TP=4 over heads. Core c handles batch b = c//4 and
global heads [4t, 4t+4) with t = c%4. Inputs are sharded/transposed on the
host with numpy; the full output y is assembled on the host from per-core
y^T slices.

Per-core SPMD program -- one software-pipelined loop over 512-col seq chunks:
  - QKV projection (bf16 matmuls, fp32 PSUM accum): chunk n of qT/kT in
    (hd, seq) layout and v in per-head (seq-block, blocks) layout (PE
    transpose), with per-partition bias adds on DVE.
  - Attention for q-blocks [4n, 4n+4) (all deps on chunks <= n): scores in
    PSUM, exp WITHOUT max-subtraction (qk dots over 128 dims are O(1); fp32
    exp cannot overflow), causal mask added in PSUM via an identity.T @
    tri(-1e30) accumulation matmul, row-sums via activation accum_out,
    P scaled by 1/l per 512-block, P transposed on PE (bf16, batched into
    512-wide PSUM tiles), P^T @ V accumulated per head -> outT (hd, seq).
  - Per (head, chunk) AllGather (groups of 4 cores) of outT in bf16.
  - Output projection y^T[n-slice] = woT^T @ gathered (bf16) + bias, emitted
    after the main loop; the Tile scheduler overlaps it with later chunks.

PSUM: one shared 4-deep pool for all (128,512)-f32 accumulators (QKV, scores,
projection) + 2 transpose banks + 2 PV banks = 8 banks exactly.
"""

import math
from contextlib import ExitStack

import numpy as np
import ml_dtypes

BF16_NP = ml_dtypes.bfloat16

import concourse.bass as bass
import concourse.mybir as mybir
import concourse.tile as tile
from concourse import bacc
from concourse.bass_utils import run_bass_kernel_spmd
from concourse.masks import make_identity

FP32 = mybir.dt.float32
FP32R = mybir.dt.float32r
BF16 = mybir.dt.bfloat16

N_CORES = 8
TP = 4  # tensor-parallel group size (heads)
HPC = 4  # heads per core
B, S, D = 2, 2048, 2048
HD = 128
NB = S // 128  # 16 seq blocks
C_SCALE = 1.0 / math.sqrt(HD)
RG = [[0, 1, 2, 3], [4, 5, 6, 7]]

_NC_CACHE = {}


def build_nc(reps: int = 1, fake_collective: bool = False):
    key = (reps, fake_collective)
    if key in _NC_CACHE:
        return _NC_CACHE[key]
    nc = bacc.Bacc("TRN2", target_bir_lowering=False, debug=False, num_devices=N_CORES)

    xT_d = nc.declare_dram_parameter("xT", [D, S], BF16, isOutput=False)
    wqkT_d = nc.declare_dram_parameter("wqkT", [D, 2 * HPC * HD], BF16, isOutput=False)
    wvT_d = nc.declare_dram_parameter("wvT", [D, HPC * HD], BF16, isOutput=False)
    bqk_d = nc.declare_dram_parameter("bqk", [128, 2 * HPC], FP32, isOutput=False)
    bv_d = nc.declare_dram_parameter("bv", [128, HPC], FP32, isOutput=False)
    woT_d = nc.declare_dram_parameter("woT", [D, HPC * HD], BF16, isOutput=False)
    bo_d = nc.declare_dram_parameter("bo", [128, HPC], FP32, isOutput=False)
    y_t_d = nc.declare_dram_parameter("y_t", [HPC * HD, S], FP32, isOutput=True)

    with tile.TileContext(nc, num_cores=N_CORES) as tc, ExitStack() as octx:
        cpool = octx.enter_context(tc.tile_pool(name="const", bufs=1))
        ident = cpool.tile([128, 128], BF16, tag="ident", name="ident")
        make_identity(nc, ident[:])
        tri_neg = cpool.tile([128, 128], BF16, tag="tri_neg", name="tri_neg")
        nc.gpsimd.memset(tri_neg[:], 0.0)
        # keep 0 where j <= p (visible), else fill -1e30 (masked)
        nc.gpsimd.affine_select(
            out=tri_neg[:], in_=tri_neg[:], pattern=[[-1, 128]],
            compare_op=mybir.AluOpType.is_ge, fill=-1e30, base=0, channel_multiplier=1,
        )
        bqk_sb = cpool.tile([128, 2 * HPC], FP32, tag="bqk", name="bqk")
        nc.sync.dma_start(out=bqk_sb[:], in_=bqk_d[:])
        bv_sb = cpool.tile([128, HPC], FP32, tag="bv", name="bv")
        nc.sync.dma_start(out=bv_sb[:], in_=bv_d[:])
        bo_sb = cpool.tile([128, HPC], FP32, tag="bo", name="bo")
        nc.sync.dma_start(out=bo_sb[:], in_=bo_d[:])

        for rep in range(reps):
            sfx = f"r{rep}"
            # per (head, seq-quarter) gather tensors, bf16
            cc_in = [[nc.dram_tensor(f"cc_in{h}_{s}_{sfx}", [HD, S // 4], BF16)
                      for s in range(4)] for h in range(HPC)]
            cc_out = [[nc.dram_tensor(f"cc_out{h}_{s}_{sfx}", [TP * HD, S // 4], BF16)
                       for s in range(4)] for h in range(HPC)]
            _body(nc, tc, xT_d, wqkT_d, wvT_d, woT_d, y_t_d,
                  bqk_sb, bv_sb, bo_sb, ident, tri_neg, cc_in, cc_out,
                  fake_collective)

    nc.compile()
    _NC_CACHE[key] = nc
    return nc


def _gather(nc, cc_in_t, cc_out_t, src_ap, fake):
    nc.sync.dma_start(out=cc_in_t[:], in_=src_ap)
    if fake:
        for rr in range(TP):
            nc.sync.dma_start(
                out=cc_out_t[rr * HD:(rr + 1) * HD, :], in_=cc_in_t[:])
    else:
        nc.gpsimd.collective_compute(
            "AllGather", mybir.AluOpType.bypass, replica_groups=RG,
            ins=[cc_in_t[:]], outs=[cc_out_t[:]])


def _body(nc, tc, xT_d, wqkT_d, wvT_d, woT_d, y_t_d,
          bqk_sb, bv_sb, bo_sb, ident, tri_neg, cc_in, cc_out,
          fake_collective=False):
    """Single software-pipelined loop over 512-col seq chunks: QKV projection
    for chunk n feeds attention for q-blocks [4n, 4n+4), whose per-quarter
    gathers feed the (program-order-later, scheduler-overlapped) output
    projection."""
    with ExitStack() as ctx:
        qkv_pool = ctx.enter_context(tc.tile_pool(name="qkv", bufs=1))
        # qT/kT per local head: (hd=128, S) bf16;  m 0-3 = q heads, 4-7 = k heads
        qkT_sb = [qkv_pool.tile([128, S], BF16, tag=f"qk{m}", name=f"qk{m}")
                  for m in range(8)]
        # v per local head: (seq-within-block=128, 16 blocks * 128) bf16
        vh_sb = [qkv_pool.tile([128, S], BF16, tag=f"vh{h}", name=f"vh{h}")
                 for h in range(HPC)]
        outT = [qkv_pool.tile([128, S], BF16, tag=f"outT{h}", name=f"outT{h}")
                for h in range(HPC)]

        wA = ctx.enter_context(tc.tile_pool(name="wA", bufs=1))
        wqk_sb = [wA.tile([128, 2 * HPC * HD], BF16, tag=f"wqk{kc}",
                          name=f"wqk{kc}") for kc in range(16)]
        wv_sb = [wA.tile([128, HPC * HD], BF16, tag=f"wv{kc}",
                         name=f"wv{kc}") for kc in range(16)]
        wo_sb = [wA.tile([128, HPC * HD], BF16, tag=f"wo{kc}", name=f"wo{kc}")
                 for kc in range(16)]

        xpool = ctx.enter_context(tc.tile_pool(name="xA", bufs=18))
        vtpool = ctx.enter_context(tc.tile_pool(name="vt", bufs=3))
        ppool = ctx.enter_context(tc.tile_pool(name="P", bufs=3))
        ptpool = ctx.enter_context(tc.tile_pool(name="pt", bufs=4))
        stat = ctx.enter_context(tc.tile_pool(name="stat", bufs=8))
        gpool = ctx.enter_context(tc.tile_pool(name="gD", bufs=17))
        ypool = ctx.enter_context(tc.tile_pool(name="yD", bufs=2))

        psW = ctx.enter_context(tc.tile_pool(name="psW", bufs=4, space="PSUM"))
        psA = psS = psD = psW  # all (128,512) f32 tiles share 4 rotating banks
        psT2 = ctx.enter_context(tc.tile_pool(name="psT2", bufs=2, space="PSUM"))
        psPV = ctx.enter_context(tc.tile_pool(name="psPV", bufs=2, space="PSUM"))

        def attention(h, qi):
            nfull = qi * 128  # cols before the diagonal block
            L = nfull + 128
            P = ppool.tile([128, L], BF16, tag="P", name="P")
            q_blk = qkT_sb[h][:, qi * 128:(qi + 1) * 128]

            ls_parts = []
            col = 0
            while col < L:
                w = min(512, L - col)
                St = psS.tile([128, w], FP32, tag="w512", name="S", padded_shape=[128, 512])
                nc.tensor.matmul(
                    St[:], q_blk, qkT_sb[HPC + h][:, col:col + w],
                    start=True, stop=(col + w <= nfull), skip_group_check=True)
                if col + w > nfull:  # chunk contains diagonal block
                    vis = nfull - col
                    # accumulate ident.T @ tri_neg = tri_neg on PE
                    nc.tensor.matmul(
                        St[:, vis:vis + 128], ident[:], tri_neg[:],
                        start=False, stop=True, skip_group_check=True)
                ls = stat.tile([128, 1], FP32, tag="ls", name="ls")
                nc.scalar.activation(
                    P[:, col:col + w], St[:],
                    mybir.ActivationFunctionType.Exp,
                    bias=0.0, scale=C_SCALE, accum_out=ls[:])
                ls_parts.append(ls)
                col += w

            lt = ls_parts[0]
            for k, extra in enumerate(ls_parts[1:]):
                lt2 = stat.tile([128, 1], FP32, tag=f"lt{k}", name=f"lt{k}")
                nc.vector.tensor_add(lt2[:], lt[:], extra[:])
                lt = lt2
            rinv = stat.tile([128, 1], FP32, tag="rinv", name="rinv")
            nc.vector.reciprocal(rinv[:], lt[:])

            pv = psPV.tile([128, 128], FP32, tag="pv", name="pv")
            nblk = qi + 1
            for g0 in range(0, nblk, 4):
                gn = min(4, nblk - g0)
                nc.vector.tensor_scalar_mul(
                    P[:, g0 * 128:(g0 + gn) * 128],
                    P[:, g0 * 128:(g0 + gn) * 128], rinv[:])
                tps = psT2.tile([128, 512], BF16, tag="tp2", name="tp2")
                for jj in range(gn):
                    nc.tensor.transpose(
                        tps[:, jj * 128:(jj + 1) * 128],
                        P[:, (g0 + jj) * 128:(g0 + jj + 1) * 128],
                        ident[:])
                ptsb = ptpool.tile([128, 512], BF16, tag="pt", name="pt")
                nc.vector.tensor_copy(ptsb[:, :gn * 128], tps[:, :gn * 128])
                for jj in range(gn):
                    j = g0 + jj
                    nc.tensor.matmul(
                        pv[:], vh_sb[h][:, j * 128:(j + 1) * 128],
                        ptsb[:, jj * 128:(jj + 1) * 128],
                        start=(j == 0), stop=(j == qi))
            nc.vector.tensor_copy(outT[h][:, qi * 128:(qi + 1) * 128], pv[:])

        for n in range(4):  # seq chunks of 512
            ncol = slice(n * 512, (n + 1) * 512)
            xts = []
            for kc in range(16):
                # interleave weight loads with the first x pass so the PE can
                # start as soon as the first chunks land
                if n == 0:
                    nc.sync.dma_start(
                        out=wqk_sb[kc][:], in_=wqkT_d[kc * 128:(kc + 1) * 128, :])
                xt = xpool.tile([128, 512], BF16, tag="xt", name="xt")
                nc.sync.dma_start(
                    out=xt[:], in_=xT_d[kc * 128:(kc + 1) * 128, ncol])
                xts.append(xt)
            if n == 0:
                for kc in range(16):
                    nc.sync.dma_start(
                        out=wv_sb[kc][:], in_=wvT_d[kc * 128:(kc + 1) * 128, :])
                for kc in range(16):
                    nc.sync.dma_start(
                        out=wo_sb[kc][:], in_=woT_d[kc * 128:(kc + 1) * 128, :])

            for m in range(12):
                psm = psA.tile([128, 512], FP32, tag="w512", name="psA")
                for kc in range(16):
                    if m < 8:
                        lhsT = wqk_sb[kc][:, m * 128:(m + 1) * 128]
                    else:
                        lhsT = wv_sb[kc][:, (m - 8) * 128:(m - 7) * 128]
                    nc.tensor.matmul(psm[:], lhsT, xts[kc][:],
                                     start=(kc == 0), stop=(kc == 15))
                if m < 8:
                    nc.vector.tensor_scalar_add(
                        qkT_sb[m][:, ncol], psm[:], bqk_sb[:, m:m + 1])
                else:
                    h = m - 8
                    vt = vtpool.tile([128, 512], BF16, tag="vt", name="vt")
                    nc.vector.tensor_scalar_add(
                        vt[:], psm[:], bv_sb[:, h:h + 1])
                    tps = psT2.tile([128, 512], BF16, tag="tp2", name="tp2")
                    for j in range(4):
                        nc.tensor.transpose(
                            tps[:, j * 128:(j + 1) * 128],
                            vt[:, j * 128:(j + 1) * 128], ident[:])
                    nc.vector.tensor_copy(vh_sb[h][:, ncol], tps[:])

            for h in range(HPC):
                for qi in range(4 * n, 4 * n + 4):
                    attention(h, qi)
                _gather(nc, cc_in[h][n], cc_out[h][n],
                        outT[h][:, n * 512:(n + 1) * 512], fake_collective)

        # ---- output projection (scheduler overlaps with later chunks) ----
        with nc.named_scope("out_proj"):
            for n in range(4):
                ncol_out = slice(n * 512, (n + 1) * 512)
                gts = []
                for kc in range(16):
                    gt = gpool.tile([128, 512], BF16, tag="gt", name="gt")
                    nc.sync.dma_start(
                        out=gt[:],
                        in_=cc_out[kc // 4][n][(kc % 4) * 128:(kc % 4 + 1) * 128, :])
                    gts.append(gt)
                for m in range(4):
                    psy = psD.tile([128, 512], FP32, tag="w512", name="py")
                    for kc in range(16):
                        nc.tensor.matmul(
                            psy[:], wo_sb[kc][:, m * 128:(m + 1) * 128],
                            gts[kc][:], start=(kc == 0), stop=(kc == 15))
                    yt = ypool.tile([128, 512], FP32, tag="yt", name="yt")
                    nc.scalar.activation(
                        yt[:], psy[:],
                        mybir.ActivationFunctionType.Identity,
                        bias=bo_sb[:, m:m + 1], scale=1.0)
                    nc.sync.dma_start(
                        out=y_t_d[m * 128:(m + 1) * 128, ncol_out], in_=yt[:])


def make_in_maps(x, w_qkv, b_qkv, w_out, b_out):
    in_maps = []
    # gathered row g = h*512 + r*128 + i  <->  w_out column (4r+h)*128 + i
    dorder = np.array(
        [(4 * r + h) * 128 + i for h in range(HPC) for r in range(TP)
         for i in range(HD)])
    for c in range(N_CORES):
        b, t = divmod(c, TP)
        xT = np.ascontiguousarray(x[b].T)
        wq = w_qkv[512 * t:512 * (t + 1)]
        wk = w_qkv[D + 512 * t:D + 512 * (t + 1)]
        wv = w_qkv[2 * D + 512 * t:2 * D + 512 * (t + 1)]
        wqkT = np.ascontiguousarray(np.concatenate([wq, wk], axis=0).T)
        wvT = np.ascontiguousarray(wv.T)
        offs_qk = [512 * t + hh * 128 for hh in range(4)] + \
                  [D + 512 * t + hh * 128 for hh in range(4)]
        bqk = np.stack([b_qkv[o:o + 128] for o in offs_qk], axis=1)
        bv = np.stack(
            [b_qkv[2 * D + 512 * t + hh * 128:2 * D + 512 * t + hh * 128 + 128]
             for hh in range(4)], axis=1)
        woT = np.ascontiguousarray(w_out[512 * t:512 * (t + 1)][:, dorder].T)
        bo = np.ascontiguousarray(b_out[512 * t:512 * (t + 1)].reshape(4, 128).T)
        in_maps.append(dict(
            xT=xT.astype(BF16_NP), wqkT=wqkT.astype(BF16_NP),
            wvT=wvT.astype(BF16_NP),
            bqk=np.ascontiguousarray(bqk), bv=np.ascontiguousarray(bv),
            woT=woT.astype(BF16_NP), bo=bo))
    return in_maps


def assemble_y(results):
    y = np.empty((B, S, D), np.float32)
    for c in range(N_CORES):
        b, t = divmod(c, TP)
        y[b][:, 512 * t:512 * (t + 1)] = results[c]["y_t"].T
    return y


def kernel(x, w_qkv, b_qkv, w_out, b_out):
    x = np.asarray(x, dtype=np.float32)
    w_qkv = np.asarray(w_qkv, dtype=np.float32)
    b_qkv = np.asarray(b_qkv, dtype=np.float32)
    w_out = np.asarray(w_out, dtype=np.float32)
    b_out = np.asarray(b_out, dtype=np.float32)

    nc = build_nc(1)
    in_maps = make_in_maps(x, w_qkv, b_qkv, w_out, b_out)
    r = run_bass_kernel_spmd(nc, in_maps, list(range(N_CORES)))
    return assemble_y(r.results)


# revision 38
# speedup vs baseline: 1.1310x; 1.0920x over previous
"""Causal self-attention (B=2, S=2048, D=2048, H=16, Hd=128) on 8 trn2 cores.

Sharding: DP=2 over batch x TP=4 over heads. Core c handles batch b = c//4 and
global heads [4t, 4t+4) with t = c%4. Inputs are sharded/transposed on the
host with numpy; the full output y is assembled on the host from per-core
y^T slices.

Per-core SPMD program -- one software-pipelined loop over 512-col seq chunks:
  - QKV projection (bf16 matmuls, fp32 PSUM accum): chunk n of qT/kT in
    (hd, seq) layout and v in per-head (seq-block, blocks) layout (PE
    transpose), with per-partition bias adds on DVE.
  - Attention for q-blocks [4n, 4n+4) (all deps on chunks <= n): scores in
    PSUM, exp WITHOUT max-subtraction (qk dots over 128 dims are O(1); fp32
    exp cannot overflow), causal mask added in PSUM via an identity.T @
    tri(-1e30) accumulation matmul, row-sums via activation accum_out,
    P scaled by 1/l per 512-block, P transposed on PE (bf16, batched into
    512-wide PSUM tiles), P^T @ V accumulated per head -> outT (hd, seq).
  - Per (head, chunk) AllGather (groups of 4 cores) of outT in bf16.
  - Output projection y^T[n-slice] = woT^T @ gathered (bf16) + bias, emitted
    after the main loop; the Tile scheduler overlaps it with later chunks.

PSUM: one shared 4-deep pool for all (128,512)-f32 accumulators (QKV, scores,
projection) + 2 transpose banks + 2 PV banks = 8 banks exactly.
"""

import math
from contextlib import ExitStack

import numpy as np
import ml_dtypes

BF16_NP = ml_dtypes.bfloat16

import concourse.bass as bass
import concourse.mybir as mybir
import concourse.tile as tile
from concourse import bacc
from concourse.bass_utils import run_bass_kernel_spmd
from concourse.masks import make_identity

FP32 = mybir.dt.float32
FP32R = mybir.dt.float32r
BF16 = mybir.dt.bfloat16

N_CORES = 8
TP = 4  # tensor-parallel group size (heads)
HPC = 4  # heads per core
B, S, D = 2, 2048, 2048
HD = 128
NB = S // 128  # 16 seq blocks
C_SCALE = 1.0 / math.sqrt(HD)
RG = [[0, 1, 2, 3], [4, 5, 6, 7]]

_NC_CACHE = {}


def build_nc(reps: int = 1, fake_collective: bool = False):
    key = (reps, fake_collective)
    if key in _NC_CACHE:
        return _NC_CACHE[key]
    nc = bacc.Bacc("TRN2", target_bir_lowering=False, debug=False, num_devices=N_CORES)

    xT_d = nc.declare_dram_parameter("xT", [D, S], BF16, isOutput=False)
    wqkT_d = nc.declare_dram_parameter("wqkT", [D, 2 * HPC * HD], BF16, isOutput=False)
    wvT_d = nc.declare_dram_parameter("wvT", [D, HPC * HD], BF16, isOutput=False)
    bqk_d = nc.declare_dram_parameter("bqk", [128, 2 * HPC], FP32, isOutput=False)
    bv_d = nc.declare_dram_parameter("bv", [128, HPC], FP32, isOutput=False)
    woT_d = nc.declare_dram_parameter("woT", [D, HPC * HD], BF16, isOutput=False)
    bo_d = nc.declare_dram_parameter("bo", [128, HPC], FP32, isOutput=False)
    y_t_d = nc.declare_dram_parameter("y_t", [HPC * HD, S], FP32, isOutput=True)

    with tile.TileContext(nc, num_cores=N_CORES) as tc, ExitStack() as octx:
        cpool = octx.enter_context(tc.tile_pool(name="const", bufs=1))
        ident = cpool.tile([128, 128], BF16, tag="ident", name="ident")
        make_identity(nc, ident[:])
        tri_neg = cpool.tile([128, 128], BF16, tag="tri_neg", name="tri_neg")
        nc.gpsimd.memset(tri_neg[:], 0.0)
        # keep 0 where j <= p (visible), else fill -1e30 (masked)
        nc.gpsimd.affine_select(
            out=tri_neg[:], in_=tri_neg[:], pattern=[[-1, 128]],
            compare_op=mybir.AluOpType.is_ge, fill=-1e30, base=0, channel_multiplier=1,
        )
        bqk_sb = cpool.tile([128, 2 * HPC], FP32, tag="bqk", name="bqk")
        nc.sync.dma_start(out=bqk_sb[:], in_=bqk_d[:])
        bv_sb = cpool.tile([128, HPC], FP32, tag="bv", name="bv")
        nc.sync.dma_start(out=bv_sb[:], in_=bv_d[:])
        bo_sb = cpool.tile([128, HPC], FP32, tag="bo", name="bo")
        nc.sync.dma_start(out=bo_sb[:], in_=bo_d[:])

        for rep in range(reps):
            sfx = f"r{rep}"
            # per (head, seq-quarter) gather tensors, bf16
            cc_in = [[nc.dram_tensor(f"cc_in{h}_{s}_{sfx}", [HD, S // 4], BF16)
                      for s in range(4)] for h in range(HPC)]
            cc_out = [[nc.dram_tensor(f"cc_out{h}_{s}_{sfx}", [TP * HD, S // 4], BF16)
                       for s in range(4)] for h in range(HPC)]
            _body(nc, tc, xT_d, wqkT_d, wvT_d, woT_d, y_t_d,
                  bqk_sb, bv_sb, bo_sb, ident, tri_neg, cc_in, cc_out,
                  fake_collective)

    nc.compile()
    _NC_CACHE[key] = nc
    return nc


def _gather(nc, cc_in_t, cc_out_t, src_ap, fake):
    nc.sync.dma_start(out=cc_in_t[:], in_=src_ap)
    if fake:
        for rr in range(TP):
            nc.sync.dma_start(
                out=cc_out_t[rr * HD:(rr + 1) * HD, :], in_=cc_in_t[:])
    else:
        nc.gpsimd.collective_compute(
            "AllGather", mybir.AluOpType.bypass, replica_groups=RG,
            ins=[cc_in_t[:]], outs=[cc_out_t[:]])


def _body(nc, tc, xT_d, wqkT_d, wvT_d, woT_d, y_t_d,
          bqk_sb, bv_sb, bo_sb, ident, tri_neg, cc_in, cc_out,
          fake_collective=False):
    """Single software-pipelined loop over 512-col seq chunks: QKV projection
    for chunk n feeds attention for q-blocks [4n, 4n+4), whose per-quarter
    gathers feed the (program-order-later, scheduler-overlapped) output
    projection."""
    with ExitStack() as ctx:
        qkv_pool = ctx.enter_context(tc.tile_pool(name="qkv", bufs=1))
        # qT/kT per local head: (hd=128, S) bf16;  m 0-3 = q heads, 4-7 = k heads
        qkT_sb = [qkv_pool.tile([128, S], BF16, tag=f"qk{m}", name=f"qk{m}")
                  for m in range(8)]
        # v per local head: (seq-within-block=128, 16 blocks * 128) bf16
        vh_sb = [qkv_pool.tile([128, S], BF16, tag=f"vh{h}", name=f"vh{h}")
                 for h in range(HPC)]
        outT = [qkv_pool.tile([128, S], BF16, tag=f"outT{h}", name=f"outT{h}")
                for h in range(HPC)]

        wA = ctx.enter_context(tc.tile_pool(name="wA", bufs=1))
        wqk_sb = [wA.tile([128, 2 * HPC * HD], BF16, tag=f"wqk{kc}",
                          name=f"wqk{kc}") for kc in range(16)]
        wv_sb = [wA.tile([128, HPC * HD], BF16, tag=f"wv{kc}",
                         name=f"wv{kc}") for kc in range(16)]
        wo_sb = [wA.tile([128, HPC * HD], BF16, tag=f"wo{kc}", name=f"wo{kc}")
                 for kc in range(16)]

        xpool = ctx.enter_context(tc.tile_pool(name="xA", bufs=18))
        vtpool = ctx.enter_context(tc.tile_pool(name="vt", bufs=3))
        ppool = ctx.enter_context(tc.tile_pool(name="P", bufs=3))
        ptpool = ctx.enter_context(tc.tile_pool(name="pt", bufs=4))
        stat = ctx.enter_context(tc.tile_pool(name="stat", bufs=8))
        gpool = ctx.enter_context(tc.tile_pool(name="gD", bufs=17))
        ypool = ctx.enter_context(tc.tile_pool(name="yD", bufs=2))

        psW = ctx.enter_context(tc.tile_pool(name="psW", bufs=4, space="PSUM"))
        psA = psS = psD = psW  # all (128,512) f32 tiles share 4 rotating banks
        psT2 = ctx.enter_context(tc.tile_pool(name="psT2", bufs=2, space="PSUM"))
        psPV = ctx.enter_context(tc.tile_pool(name="psPV", bufs=2, space="PSUM"))

        def attention(h, qi):
            nfull = qi * 128  # cols before the diagonal block
            L = nfull + 128
            P = ppool.tile([128, L], BF16, tag="P", name="P")
            q_blk = qkT_sb[h][:, qi * 128:(qi + 1) * 128]

            ls_parts = []
            col = 0
            while col < L:
                w = min(512, L - col)
                St = psS.tile([128, w], FP32, tag="w512", name="S", padded_shape=[128, 512])
                nc.tensor.matmul(
                    St[:], q_blk, qkT_sb[HPC + h][:, col:col + w],
                    start=True, stop=(col + w <= nfull), skip_group_check=True)
                if col + w > nfull:  # chunk contains diagonal block
                    vis = nfull - col
                    # accumulate ident.T @ tri_neg = tri_neg on PE
                    nc.tensor.matmul(
                        St[:, vis:vis + 128], ident[:], tri_neg[:],
                        start=False, stop=True, skip_group_check=True)
                ls = stat.tile([128, 1], FP32, tag="ls", name="ls")
                nc.scalar.activation(
                    P[:, col:col + w], St[:],
                    mybir.ActivationFunctionType.Exp,
                    bias=0.0, scale=C_SCALE, accum_out=ls[:])
                ls_parts.append(ls)
                col += w

            lt = ls_parts[0]
            for k, extra in enumerate(ls_parts[1:]):
                lt2 = stat.tile([128, 1], FP32, tag=f"lt{k}", name=f"lt{k}")
                nc.vector.tensor_add(lt2[:], lt[:], extra[:])
                lt = lt2
            rinv = stat.tile([128, 1], FP32, tag="rinv", name="rinv")
            nc.vector.reciprocal(rinv[:], lt[:])

            pv = psPV.tile([128, 128], FP32, tag="pv", name="pv")
            nblk = qi + 1
            for g0 in range(0, nblk, 4):
                gn = min(4, nblk - g0)
                nc.vector.tensor_scalar_mul(
                    P[:, g0 * 128:(g0 + gn) * 128],
                    P[:, g0 * 128:(g0 + gn) * 128], rinv[:])
                tps = psT2.tile([128, 512], BF16, tag="tp2", name="tp2")
                for jj in range(gn):
                    nc.tensor.transpose(
                        tps[:, jj * 128:(jj + 1) * 128],
                        P[:, (g0 + jj) * 128:(g0 + jj + 1) * 128],
                        ident[:])
                ptsb = ptpool.tile([128, 512], BF16, tag="pt", name="pt")
                nc.vector.tensor_copy(ptsb[:, :gn * 128], tps[:, :gn * 128])
                for jj in range(gn):
                    j = g0 + jj
                    nc.tensor.matmul(
                        pv[:], vh_sb[h][:, j * 128:(j + 1) * 128],
                        ptsb[:, jj * 128:(jj + 1) * 128],
                        start=(j == 0), stop=(j == qi))
            nc.vector.tensor_copy(outT[h][:, qi * 128:(qi + 1) * 128], pv[:])

        for n in range(4):  # seq chunks of 512
            ncol = slice(n * 512, (n + 1) * 512)
            xts = []
            for kc in range(16):
                # interleave weight loads with the first x pass so the PE can
                # start as soon as the first chunks land
                if n == 0:
                    nc.sync.dma_start(
                        out=wqk_sb[kc][:], in_=wqkT_d[kc * 128:(kc + 1) * 128, :])
                xt = xpool.tile([128, 512], BF16, tag="xt", name="xt")
                nc.sync.dma_start(
                    out=xt[:], in_=xT_d[kc * 128:(kc + 1) * 128, ncol])
                xts.append(xt)
            if n == 0:
                for kc in range(16):
                    nc.sync.dma_start(
                        out=wv_sb[kc][:], in_=wvT_d[kc * 128:(kc + 1) * 128, :])
                for kc in range(16):
                    nc.sync.dma_start(
                        out=wo_sb[kc][:], in_=woT_d[kc * 128:(kc + 1) * 128, :])

            for m in range(12):
                psm = psA.tile([128, 512], FP32, tag="w512", name="psA")
                for kc in range(16):
                    if m < 8:
                        lhsT = wqk_sb[kc][:, m * 128:(m + 1) * 128]
                    else:
                        lhsT = wv_sb[kc][:, (m - 8) * 128:(m - 7) * 128]
                    nc.tensor.matmul(psm[:], lhsT, xts[kc][:],
                                     start=(kc == 0), stop=(kc == 15))
                if m < 8:
                    nc.vector.tensor_scalar_add(
                        qkT_sb[m][:, ncol], psm[:], bqk_sb[:, m:m + 1])
                else:
                    h = m - 8
                    vt = vtpool.tile([128, 512], BF16, tag="vt", name="vt")
                    nc.vector.tensor_scalar_add(
                        vt[:], psm[:], bv_sb[:, h:h + 1])
                    tps = psT2.tile([128, 512], BF16, tag="tp2", name="tp2")
                    for j in range(4):
                        nc.tensor.transpose(
                            tps[:, j * 128:(j + 1) * 128],
                            vt[:, j * 128:(j + 1) * 128], ident[:])
                    nc.vector.tensor_copy(vh_sb[h][:, ncol], tps[:])

            for h in range(HPC):
                for qi in range(4 * n, 4 * n + 4):
                    attention(h, qi)
                _gather(nc, cc_in[h][n], cc_out[h][n],
                        outT[h][:, n * 512:(n + 1) * 512], fake_collective)

        # ---- output projection (scheduler overlaps with later chunks) ----
        with nc.named_scope("out_proj"):
            for n in range(4):
                ncol_out = slice(n * 512, (n + 1) * 512)
                gts = []
                for kc in range(16):
                    gt = gpool.tile([128, 512], BF16, tag="gt", name="gt")
                    nc.sync.dma_start(
                        out=gt[:],
                        in_=cc_out[kc // 4][n][(kc % 4) * 128:(kc % 4 + 1) * 128, :])
                    gts.append(gt)
                for m in range(4):
                    psy = psD.tile([128, 512], FP32, tag="w512", name="py")
                    for kc in range(16):
                        nc.tensor.matmul(
                            psy[:], wo_sb[kc][:, m * 128:(m + 1) * 128],
                            gts[kc][:], start=(kc == 0), stop=(kc == 15))
                    yt = ypool.tile([128, 512], FP32, tag="yt", name="yt")
                    nc.scalar.activation(
                        yt[:], psy[:],
                        mybir.ActivationFunctionType.Identity,
                        bias=bo_sb[:, m:m + 1], scale=1.0)
                    nc.sync.dma_start(
                        out=y_t_d[m * 128:(m + 1) * 128, ncol_out], in_=yt[:])


def make_in_maps(x, w_qkv, b_qkv, w_out, b_out):
    in_maps = []
    # gathered row g = h*512 + r*128 + i  <->  w_out column (4r+h)*128 + i
    dorder = np.array(
        [(4 * r + h) * 128 + i for h in range(HPC) for r in range(TP)
         for i in range(HD)])
    for c in range(N_CORES):
        b, t = divmod(c, TP)
        xT = np.ascontiguousarray(x[b].T)
        wq = w_qkv[512 * t:512 * (t + 1)]
        wk = w_qkv[D + 512 * t:D + 512 * (t + 1)]
        wv = w_qkv[2 * D + 512 * t:2 * D + 512 * (t + 1)]
        wqkT = np.ascontiguousarray(np.concatenate([wq, wk], axis=0).T)
        wvT = np.ascontiguousarray(wv.T)
        offs_qk = [512 * t + hh * 128 for hh in range(4)] + \
                  [D + 512 * t + hh * 128 for hh in range(4)]
        bqk = np.stack([b_qkv[o:o + 128] for o in offs_qk], axis=1)
        bv = np.stack(
            [b_qkv[2 * D + 512 * t + hh * 128:2 * D + 512 * t + hh * 128 + 128]
             for hh in range(4)], axis=1)
        woT = np.ascontiguousarray(w_out[512 * t:512 * (t + 1)][:, dorder].T)
        bo = np.ascontiguousarray(b_out[512 * t:512 * (t + 1)].reshape(4, 128).T)
        in_maps.append(dict(
            xT=xT.astype(BF16_NP), wqkT=wqkT.astype(BF16_NP),
            wvT=wvT.astype(BF16_NP),
            bqk=np.ascontiguousarray(bqk), bv=np.ascontiguousarray(bv),
            woT=woT.astype(BF16_NP), bo=bo))
    return in_maps


def assemble_y(results):
    y = np.empty((B, S, D), np.float32)
    for c in range(N_CORES):
        b, t = divmod(c, TP)
        y[b][:, 512 * t:512 * (t + 1)] = results[c]["y_t"].T
    return y


def kernel(x, w_qkv, b_qkv, w_out, b_out):
    x = np.asarray(x, dtype=np.float32)
    w_qkv = np.asarray(w_qkv, dtype=np.float32)
    b_qkv = np.asarray(b_qkv, dtype=np.float32)
    w_out = np.asarray(w_out, dtype=np.float32)
    b_out = np.asarray(b_out, dtype=np.float32)

    nc = build_nc(1)
    in_maps = make_in_maps(x, w_qkv, b_qkv, w_out, b_out)
    r = run_bass_kernel_spmd(nc, in_maps, list(range(N_CORES)))
    return assemble_y(r.results)
